# revision 35
# baseline (speedup 1.0000x reference)
"""Multi-head attention (B=4, S=2048, D=1024, H=16) on 8 TRN2 NeuronCores.

Sharding: data-parallel over batch (4) x tensor-parallel over head halves (2).
Core c handles batch b = c//2 and heads [8g, 8g+8) where g = c%2.
Each core computes a partial [S, D] output-projection contribution; the host
sums the two head-group partials per batch.

Layout (all matmul operands bf16, fp32 PSUM accumulation):
  - activations arrive host-transposed (feature dim on partitions),
  - scores are built transposed [k, q]; one PSUM tile [128, 1024] holds the
    scores of BOTH heads of a pair ([k, head0 q | head1 q]) so the two
    DK=64 score matmuls (PE row halves 0-63 / 64-127, tile_position (0,0)
    and (64,0)) become ready together, schedule back-to-back, and run
    CONCURRENTLY on the PE array (row tiling),
  - exp() covers both heads in one [128, 1024] activation instruction,
  - P @ [V | 1] yields the softmax denominator as row 64 of U^T for free,
  - normalized U^T tiles are exactly the stationary layout w_o needs.

Scheduling: the attention stream (score MMs -> exp -> PV MMs) is emitted at
normal priority; every projection/output-projection "piece" is emitted at
LOW priority (tc.high_priority with a negative offset) so the Tile
scheduler treats pieces as pure PE gap-fillers — the scalar engine's exp
stream (~266us busy) never waits behind piece work.

Phase A processes q-chunks {0,1} interleaved c-major so the K/V projection
of chunks 1-3 fits under 2x of exp cover; phase B runs q-chunks 2,3
qc-major with the w_o pieces of earlier chunks as fillers. The final
chunk's w_o runs at quarter width (N=256) to shorten the serial tail.

PSUM budget (8 banks): 2 x sc [128,1024] (4 banks) + 2 x Up [65,512]
(2 banks) + 2 x piece accumulators [128,512] (2 banks).
"""

import numpy as np

B, S, D, H = 4, 2048, 1024, 16
DK = D // H          # 64
G = 2                # head groups (tensor-parallel degree per batch)
HL = H // G          # 8 local heads per core
DV = HL * DK         # 512 local value dim
N_CORES = 8

LOWP = -1_000_000    # priority offset for gap-filler pieces

_cached = {}


def _build():
    import concourse.bass as bass
    import concourse.tile as tile
    from concourse import bacc, mybir

    f32 = mybir.dt.float32
    bf16 = mybir.dt.bfloat16
    EXP = mybir.ActivationFunctionType.Exp

    nc = bacc.Bacc("TRN2", target_bir_lowering=False, debug=False,
                   num_devices=N_CORES)

    scr = nc.dram_tensor("scr", [128, 512], f32, kind="Internal").ap()
    xqT = nc.dram_tensor("xqT", [S // 512, 128, D // 128, 512], bf16,
                         kind="ExternalInput").ap()
    xkT = nc.dram_tensor("xkT", [S // 512, 128, D // 128, 512], bf16,
                         kind="ExternalInput").ap()
    xvT = nc.dram_tensor("xvT", [S // 512, 128, D // 128, 512], bf16,
                         kind="ExternalInput").ap()
    wqT = nc.dram_tensor("wqT", [128, D // 128, DV], bf16,
                         kind="ExternalInput").ap()
    wkT = nc.dram_tensor("wkT", [128, D // 128, DV], bf16,
                         kind="ExternalInput").ap()
    wvT = nc.dram_tensor("wvT", [128, D // 128, DV], bf16,
                         kind="ExternalInput").ap()
    woT = nc.dram_tensor("woT", [DV, D], bf16, kind="ExternalInput").ap()
    out = nc.dram_tensor("out", [S, D], f32, kind="ExternalOutput").ap()

    ND = D // 128     # 8 d-tiles
    NS = S // 128     # 16 k-tiles
    NQC = S // 512    # 4 q-chunks
    NT = DV // 128    # 4 dk/dv-tiles
    NHP = HL // 2     # 4 head pairs

    with tile.TileContext(nc) as tc:
        with (
            tc.tile_pool(name="persist", bufs=1) as persist,
            tc.tile_pool(name="stage", bufs=28) as stage,
            tc.tile_pool(name="wpool", bufs=8) as wpool,
            tc.tile_pool(name="spool", bufs=2, space=bass.MemorySpace.PSUM) as spool,
            tc.tile_pool(name="upool", bufs=2, space=bass.MemorySpace.PSUM) as upool,
            tc.tile_pool(name="gpool", bufs=2, space=bass.MemorySpace.PSUM) as gpool,
            tc.tile_pool(name="ppool", bufs=3) as ppool,
            tc.tile_pool(name="rpool", bufs=3) as rpool,
            tc.tile_pool(name="obuf", bufs=3) as obuf,
        ):
            QT = {}    # [t][qc] -> [128, 512] tiles of Q^T (dk rows, q cols)
            KT = {}    # [t][c]  -> [128, 512]
            Vaug = {}  # [kt] -> [128, 8, 65]: per-head V columns + ones col
            outT = {}  # [qc][t] -> [128, 512] normalized attention out^T
            wos = []
            st_ = {}   # per (qc, hp) attention state
            xq_stage = {}

            def uacc():
                return upool.tile([65, 512], f32, tag="u", name="uacc")

            def gacc():
                return gpool.tile([128, 512], f32, tag="g", name="gacc")

            def emit_w_load(name, src):
                # one fused DMA per weight tensor: [128, ND, 512] tile from
                # a host-tiled contiguous block (8 KB partition lines).
                wt = wpool.tile([128, ND, DV], bf16, tag="w", name=name,
                                bufs=3)
                h = ND // 2
                nc.sync.dma_start(wt[:, 0:h, :], src[:, 0:h, :])
                nc.sync.dma_start(wt[:, h:ND, :], src[:, h:ND, :])
                return wt

            def emit_wo_loads():
                for t in range(NT):
                    wo = wpool.tile([128, D], bf16, tag=f"wo{t}", name="wo",
                                    bufs=1)
                    nc.sync.dma_start(wo[:], woT[128 * t:128 * (t + 1), :])
                    wos.append(wo)

            def emit_x_dmas(src, c, name):
                xt = stage.tile([128, ND, 512], bf16, tag="xkv", name=name,
                                bufs=3)
                h = ND // 2
                nc.sync.dma_start(xt[:, 0:h, :], src[c, :, 0:h, :])
                nc.sync.dma_start(xt[:, h:ND, :], src[c, :, h:ND, :])
                return xt

            def piece_kproj(xks, c, t):
                def go():
                    acc = gacc()
                    for d in range(ND):
                        nc.tensor.matmul(
                            acc[:], wkt[:, d, 128 * t:128 * (t + 1)],
                            xks[:, d, :],
                            start=(d == 0), stop=(d == ND - 1))
                    dt_ = persist.tile([128, 512], bf16, tag=f"kT{t}_{c}",
                                       name="kT")
                    nc.vector.tensor_copy(dt_[:], acc[:])
                    KT.setdefault(t, {})[c] = dt_
                return go

            def piece_vproj(xvs, c, ktl):
                def go():
                    kt = 4 * c + ktl
                    acc = gacc()
                    for d in range(ND):
                        nc.tensor.matmul(
                            acc[:], xvs[:, d, 128 * ktl:128 * (ktl + 1)],
                            wvt[:, d, :],
                            start=(d == 0), stop=(d == ND - 1))
                    va = persist.tile([128, HL, DK + 1], bf16, tag=f"vaug{kt}",
                                      name="vaug")
                    nc.vector.tensor_copy(
                        va[:, :, 0:DK],
                        acc[:].rearrange("p (h k) -> p h k", h=HL))
                    nc.vector.tensor_copy(
                        va[:, :, DK], nc.const_aps.tensor(1.0, (128, HL), bf16))
                    Vaug[kt] = va
                return go

            def emit_xq_dmas(qc):
                xt = stage.tile([128, ND, 512], bf16, tag="xq", name="xq",
                                bufs=2)
                nc.sync.dma_start(xt[:], xqT[qc, :, :, :])
                xq_stage[qc] = xt

            def piece_qproj(qc, t):
                def go():
                    xs = xq_stage[qc]
                    acc = gacc()
                    for d in range(ND):
                        nc.tensor.matmul(
                            acc[:], wqt[:, d, 128 * t:128 * (t + 1)],
                            xs[:, d, :],
                            start=(d == 0), stop=(d == ND - 1))
                    dt_ = persist.tile([128, 512], bf16, tag=f"qT{t}_{qc}",
                                       name="qT")
                    nc.vector.tensor_copy(dt_[:], acc[:])
                    QT.setdefault(t, {})[qc] = dt_
                return go

            wo_stash = {}

            def piece_wo(qc, st, ncols, ts=None, mode="full", upacc=False):
                # final[s, n] = sum_dv outT[dv, s] * woT[dv, n].  The final
                # chunk is staged: mode="init" sums head tiles {0,1} into an
                # SBUF stash as soon as their normalize lands, "add" folds
                # further tiles in, "final" folds the last tile and stores —
                # only one matmul layer remains after the last normalize.
                def go():
                    acc = (upool.tile([128, 512], f32, tag="u", name="uacc")
                           if upacc else gacc())
                    tl = list(range(NT) if ts is None else ts)
                    for j, t in enumerate(tl):
                        nc.tensor.matmul(
                            acc[:],
                            outT[qc][t][:, 128 * st:128 * (st + 1)],
                            wos[t][:, 512 * ncols:512 * (ncols + 1)],
                            start=(j == 0), stop=(j == len(tl) - 1))
                    if mode == "init":
                        sb = obuf.tile([128, 512], f32, tag="ob",
                                       name="ob", bufs=8)
                        nc.vector.tensor_copy(sb[:], acc[:])
                        wo_stash[(qc, st, ncols)] = sb
                        return
                    if mode in ("add", "final"):
                        ob = wo_stash[(qc, st, ncols)]
                        nc.vector.tensor_add(ob[:], ob[:], acc[:])
                        if mode == "add":
                            return
                    else:
                        ob = obuf.tile([128, 512], f32, tag="ob", name="ob",
                                       bufs=8)
                        nc.vector.tensor_copy(ob[:], acc[:])
                    nc.sync.dma_start(
                        out[512 * qc + 128 * st:512 * qc + 128 * (st + 1),
                            512 * ncols:512 * (ncols + 1)],
                        ob[:])
                return go

            def lowp(fn):
                def go():
                    with tc.high_priority(offset=LOWP):
                        fn()
                return go

            def emit_attn_turn(qc, hp, c, accum16=False):
                # head pair (2hp, 2hp+1) = partition halves of tile hp. One
                # sc tile [128 kpos, head0 q | head1 q] per k-tile: the two
                # DK=64 score matmuls share the tile (ready together ->
                # adjacent in the PE queue -> concurrent row tiles).
                # PV matmuls are emitted AFTER the next k-tile's score pair
                # (software pipelining): the score pair then outranks a
                # just-became-ready PV in the scheduler, so pairs stay
                # adjacent and PV never parks the PE queue on its exp.
                t = hp
                s = st_.setdefault((qc, hp), {})
                if not accum16 and c == 0:
                    s["Usb"] = [rpool.tile([65, 512], f32, tag=f"usb{hp}_{i}",
                                           name="usb", bufs=2)
                                for i in range(2)]
                if accum16:
                    if c == 0:
                        s["Up"] = [uacc(), uacc()]
                    Up = s["Up"]
                else:
                    Up = [None, None]
                pend = []

                def flush_pv():
                    P, kt = pend.pop(0)
                    ktl = kt - 4 * c
                    st_first = kt == 0 if accum16 else ktl == 0
                    st_last = kt == NS - 1 if accum16 else ktl == 3
                    for i in range(2):
                        nc.tensor.matmul(
                            Up[i][:],
                            Vaug[kt][:, 2 * hp + i, :],
                            P[:, 512 * i:512 * (i + 1)],
                            start=st_first, stop=st_last)

                for ktl in range(4):
                    kt = 4 * c + ktl
                    sc = spool.tile([128, 1024], f32, tag="sc", name="sc")
                    for i in range(2):
                        po = 64 * i
                        nc.tensor.matmul(
                            sc[:, 512 * i:512 * (i + 1)],
                            KT[t][c][po:po + 64,
                                     128 * ktl:128 * (ktl + 1)],
                            QT[t][qc][po:po + 64, :],
                            start=True, stop=True)
                    P = ppool.tile([128, 1024], bf16, tag="p", name="p")
                    nc.scalar.activation(P[:], sc[:], EXP, scale=0.125)
                    if not accum16 and ktl == 0:
                        Up[0] = uacc()
                        Up[1] = uacc()
                    pend.append((P, kt))
                    if ktl >= 1:
                        flush_pv()
                flush_pv()
                if not accum16:
                    for i in range(2):
                        if c == 0:
                            nc.vector.tensor_copy(s["Usb"][i][:], Up[i][:])
                        else:
                            nc.vector.tensor_add(s["Usb"][i][:],
                                                 s["Usb"][i][:], Up[i][:])
                elif c == 3:
                    # single fold: frees the Up PSUM pair with one fast
                    # copy instead of holding it through the normalize
                    # reciprocal chain.
                    s["Usb"] = [rpool.tile([65, 512], f32, tag=f"usb{hp}_{i}",
                                           name="usb", bufs=2)
                                for i in range(2)]
                    for i in range(2):
                        nc.vector.tensor_copy(s["Usb"][i][:], Up[i][:])

            def emit_normalize(qc, hp, from_psum=False):
                # rows 0..63 of U divided by row 64 (the ones-column sum),
                # written into out^T. Engine ops keep operands on one
                # partition range; cross-partition moves via SBUF-SBUF DMA
                # (which cannot read PSUM, hence the row-64 extract copy).
                t = hp
                s = st_[(qc, hp)]
                src = s["Up"] if from_psum else s["Usb"]
                ot = persist.tile([128, 512], bf16, tag=f"oT{t}_{qc % 2}",
                                  name="oT")
                outT.setdefault(qc, {})[t] = ot
                # the two head sub-chains are independent: emit stage-by-
                # stage so their latencies overlap (DVE/GpSimd are FIFO).
                rrecs, rbs = [], []
                for i in range(2):
                    rrow = rpool.tile([1, 512], f32, tag="rrow", name="rrow")
                    nc.sync.dma_start(rrow[:], src[i][64:65, :])
                    rrec = rpool.tile([1, 512], f32, tag="rrec", name="rrec")
                    nc.vector.reciprocal_approx_fast(rrec[:], rrow[:])
                    rrecs.append(rrec)
                for i in range(2):
                    rb = rpool.tile([64, 512], f32, tag="rb", name="rb",
                                    bufs=2)
                    nc.gpsimd.partition_broadcast(rb[:], rrecs[i][:])
                    rbs.append(rb)
                for i in range(2):
                    if i == 0:
                        nc.vector.tensor_mul(ot[0:64, :], src[i][0:64, :],
                                             rbs[i][:])
                    else:
                        stg = rpool.tile([64, 512], bf16, tag="stg",
                                         name="stg", bufs=2)
                        nc.vector.tensor_mul(stg[:], src[i][0:64, :],
                                             rbs[i][:])
                        nc.sync.dma_start(ot[64:128, :], stg[:])

            # ---- warm-up: load the exp table + flip the PE HAM to full
            # clock during the initial DMA wait, using a zeroed SBUF tile.
            # The exp reads the warm matmul's PSUM and the result lands in
            # `out` (overwritten later) so nothing here is dead code.
            wtile = stage.tile([128, 512], bf16, tag="warm", name="warm",
                               bufs=1)
            nc.vector.memset(wtile[:], 0.0)
            wacc = gacc()
            for r in range(16):
                nc.tensor.matmul(wacc[:], wtile[:, 0:128], wtile[:],
                                 start=(r == 0), stop=(r == 15))
            wexp = stage.tile([128, 512], f32, tag="warm2", name="warm2",
                              bufs=1)
            nc.scalar.activation(wexp[:], wacc[:], EXP, scale=0.125)

            # ---- DMA order: V path first (first PE work), then K, Q —
            # six fused 1 MB transfers cover the whole prologue.
            wvt = emit_w_load("wv", wvT)
            xvs0 = emit_x_dmas(xvT, 0, "xv")
            wkt = emit_w_load("wk", wkT)
            xks0 = emit_x_dmas(xkT, 0, "xk")
            wqt = emit_w_load("wq", wqT)
            emit_xq_dmas(0)
            emit_xq_dmas(1)

            xstage = {0: (xks0, xvs0)}

            # Pieces keyed for just-in-time emission before the turn that
            # first reads their tile (a Python-level ordering requirement;
            # execution order is still dependency + priority driven).
            emitted = set()

            def emit_piece(key):
                if key in emitted:
                    return
                emitted.add(key)
                kind = key[0]
                if kind == "v":
                    _, c, ktl = key
                    lowp(piece_vproj(xstage[c][1], c, ktl))()
                elif kind == "k":
                    _, c, t = key
                    lowp(piece_kproj(xstage[c][0], c, t))()
                elif kind == "q":
                    _, qc, t = key
                    lowp(piece_qproj(qc, t))()

            def turn_needs(qc, hp, c):
                # kproj first: at a chunk boundary the scores of the next
                # chunk only need K^T, so it should outrank the V pieces.
                return ([("k", c, hp), ("q", qc, hp)]
                        + [("v", c, ktl) for ktl in range(4)])

            # ---- prologue: minimal deps of the first attention turn at
            # normal priority; the other three Vaug tiles follow low-prio.
            piece_vproj(xvs0, 0, 0)()
            piece_kproj(xks0, 0, 0)()
            piece_qproj(0, 0)()
            emitted |= {("v", 0, 0), ("k", 0, 0), ("q", 0, 0)}

            # ---- phase A: q-chunks {0, 1} interleaved, c-major, so the
            # K/V projection of chunks 1-3 sits under 2x of exp cover.
            extras = [("q", 2, t) for t in range(NT)]
            for c in range(4):
                if c < 3:
                    xstage[c + 1] = (emit_x_dmas(xkT, c + 1, "xk"),
                                     emit_x_dmas(xvT, c + 1, "xv"))
                if c == 1:
                    emit_wo_loads()
                    emit_xq_dmas(2)
                    emit_xq_dmas(3)
                for hp in range(NHP):
                    for qc in (0, 1):
                        for key in turn_needs(qc, hp, c):
                            emit_piece(key)
                        emit_attn_turn(qc, hp, c)
                        if c == 3:
                            emit_normalize(qc, hp)
                        if c >= 2 and extras:
                            emit_piece(extras.pop(0))

            # ---- phase B: q-chunks 2 then 3, hp-major (a head pair's
            # normalize lands as soon as its four c-groups finish, so the
            # final chunk's w_o half-pieces over head tiles {0,1} overlap
            # the remaining attention); w_o of earlier chunks as fillers.
            extras = [("q", 3, t) for t in range(NT)]
            for qc in (2, 3):
                for hp in range(NHP):
                    emit_piece(("q", qc, hp))
                    for c in range(4):
                        emit_attn_turn(qc, hp, c, accum16=True)
                        if c == 3:
                            emit_normalize(qc, hp)
                        if qc == 2 and extras:
                            emit_piece(extras.pop(0))
                    if qc == 3 and hp in (1, 2):
                        # stage the final chunk's w_o as each head pair's
                        # normalize lands.
                        for st2 in range(4):
                            for ncol in range(2):
                                if hp == 1:
                                    lowp(piece_wo(3, st2, ncol, (0, 1),
                                                  "init"))()
                                else:
                                    lowp(piece_wo(3, st2, ncol, (2,),
                                                  "add"))()
                if qc == 2:
                    for wqc in (0, 1, 2):
                        for st2 in range(4):
                            for ncol in range(2):
                                lowp(piece_wo(wqc, st2, ncol))()
            for j, (st2, ncol) in enumerate(
                    (s2, n2) for s2 in range(4) for n2 in range(2)):
                lowp(piece_wo(3, st2, ncol, (3,), "final", upacc=j % 2))()

            # warm-exp sink: a scratch-DRAM store, last in every queue.
            with tc.high_priority(offset=LOWP * 2):
                nc.sync.dma_start(scr[:, :], wexp[:])

    nc.compile()
    return nc


def kernel(query, key, value, w_q, w_k, w_v, w_o):
    import ml_dtypes
    from concourse.bass_utils import run_bass_kernel_spmd

    if "nc" not in _cached:
        _cached["nc"] = _build()
    nc = _cached["nc"]

    bf = ml_dtypes.bfloat16
    query = np.asarray(query, dtype=np.float32)
    key = np.asarray(key, dtype=np.float32)
    value = np.asarray(value, dtype=np.float32)
    w_q = np.asarray(w_q, dtype=np.float32)
    w_k = np.asarray(w_k, dtype=np.float32)
    w_v = np.asarray(w_v, dtype=np.float32)
    w_o = np.asarray(w_o, dtype=np.float32)

    def c(a):
        return np.ascontiguousarray(a).astype(bf)

    def xtile(a):  # [S, D] activations -> [4, 128, 8, 512] chunk-contiguous
        return c(a.T.reshape(8, 128, 4, 512).transpose(2, 1, 0, 3))

    def wtile(a):  # [D, DV] weightsT -> [128, 8, 512] partition-contiguous
        return c(a.reshape(8, 128, DV).transpose(1, 0, 2))

    in_maps = []
    for core in range(N_CORES):
        b, g = core // G, core % G
        rows = slice(DV * g, DV * (g + 1))
        in_maps.append({
            "xqT": xtile(query[b]),
            "xkT": xtile(key[b]),
            "xvT": xtile(value[b]),
            "wqT": wtile(w_q[rows, :].T),
            "wkT": wtile(w_k[rows, :].T),
            "wvT": wtile(w_v[rows, :].T),
            "woT": c(w_o[:, rows].T),
        })

    res = run_bass_kernel_spmd(nc, in_maps, list(range(N_CORES)))
    full = np.empty((B, S, D), np.float32)
    for b in range(B):
        full[b] = res.results[G * b]["out"] + res.results[G * b + 1]["out"]
    return full
